# revision 1
# baseline (speedup 1.0000x reference)
"""Causal full-d_model attention (B=4, T=2048, C=1024) on 8 Trainium2 cores.

Sharding: core = 2*b + p handles batch b and two 512-row sequence blocks,
p=0 -> blocks {0, 3}, p=1 -> blocks {1, 2} (pairing balances causal work).
K/V projections for the full sequence are computed redundantly on both
cores of a batch pair; the causal skip of upper-triangle score/PV tiles
pays back exactly that duplication, so per-core FLOPs equal an ideal
8-way split (~17.2 GFLOP).

On-device layout is transposed ([feature, token]) so every matmul
contracts along the partition axis:
    qT/kT = W.T @ xT                       (projection)
    scoresT[j, i] = kT_slice.T @ qT        (j on partitions)
    attnT[c, i]  += v_slice.T @ probsT     (accumulate over j tiles)
    outT          = Wo_slice.T @ attnT
All matmul operands are float32r (single-pass PE mode, 4x the fp32
LOW_HIGH rate at moving dims >= 256; measured rel err ~3e-4 vs the
fp32 reference); accumulation is fp32 in PSUM and the softmax epilogue
(denominator, reciprocal, biases) stays fp32.
Softmax is unnormalized (no max subtraction: scores ~ N(0,1), so exp is
safe); the denominator comes from an M=1 ones-column matmul over the
masked exp tiles and is applied to attnT at the PSUM->SBUF copy.
Causal masks arrive as per-core input data (a shifted window into a
master 0/1 pattern), so all 8 cores run one SPMD program even though
their absolute row offsets differ. v is staged through internal DRAM;
kT/qT stay resident in SBUF.
"""

import math

import numpy as np

P = 128          # SBUF partitions
B_, T_, C_ = 4, 2048, 1024


def _emit(nc, tc, aps, T, C):
    import concourse.bass as bass
    from concourse import mybir
    from concourse.tile_rust import add_dep_helper
    from contextlib import ExitStack

    AFT = mybir.ActivationFunctionType
    f32 = mybir.dt.float32
    f32r = mybir.dt.float32r

    NT = C // P            # feature tiles
    BLK = T // 4           # sequence block (also i-slot width FB)
    TL = 2 * BLK           # local query tokens per core
    FB = BLK               # matmul moving free dim for i
    assert FB <= 512
    FBC = min(256, T)      # x chunk width (K/V passes)
    NCHK = T // FBC
    FBQ = min(256, TL)     # xq chunk width (Q pass)
    CH = min(512, C)       # v c_out chunk
    NCH = C // CH
    njA = (2 * BLK) // P   # padded j-tiles for slot A
    njB = (4 * BLK) // P   # padded j-tiles for slot B
    CA = P * (njA - 1)
    NJB0 = njA            # slot-B j-tiles < NJB0 are causally all-ones for
    CB = P * (njB - 1 - NJB0)   # every core; mask multiply is skipped there
    SC = 1.0 / math.sqrt(C)

    (xTb, xq, Wk, Wv, Wq, WoT, bq_t, bk_t, bo_t, ones_d, mA, mB, v_dram,
     rec_dram, outT) = aps

    with ExitStack() as ctx:
        singles = ctx.enter_context(tc.tile_pool(name="singles", bufs=1))
        kpool = ctx.enter_context(tc.tile_pool(name="kpool", bufs=1))
        qpool = ctx.enter_context(tc.tile_pool(name="qpool", bufs=1))
        psp = ctx.enter_context(tc.tile_pool(name="psp", bufs=8, space="PSUM"))

        bq_sb = singles.tile([P, NT], f32, name="bq_sb")
        bk_sb = singles.tile([P, NT], f32, name="bk_sb")
        bo_sb = singles.tile([P, NT], f32, name="bo_sb")
        ones_sb = singles.tile([P, 1], f32r, name="ones_sb")
        nc.sync.dma_start(out=bq_sb, in_=bq_t)
        nc.sync.dma_start(out=bk_sb, in_=bk_t)
        nc.sync.dma_start(out=bo_sb, in_=bo_t)
        nc.sync.dma_start(out=ones_sb, in_=ones_d)

        kT_sb = kpool.tile([P, NT, T], f32r, name="kT_sb")
        qT_sb = qpool.tile([P, NT, TL], f32r, name="qT_sb")
        v_w_insts = {}  # global j-tile -> DMA write insts (DRAM RAW edges)

        maskp = ctx.enter_context(tc.tile_pool(name="maskp", bufs=1))
        mA_sb = maskp.tile([P, CA + FB], f32r, name="mA_sb")
        mB_sb = maskp.tile([P, CB + FB], f32r, name="mB_sb")

        # ------- phase 1: K+V fused over x chunks, then Q -------
        with ExitStack() as p1:
          with ExitStack() as pkv0:
            wpool = pkv0.enter_context(tc.tile_pool(name="wpool", bufs=2))

            wk_sb = wpool.tile([P, NT, C], f32r, name="w_sb", tag="w")
            for co in range(NT):
                nc.sync.dma_start(
                    out=wk_sb[:, :, co * P:(co + 1) * P],
                    in_=Wk[co].rearrange("(ci p) m -> p ci m", p=P),
                )
            wv_sb = wpool.tile([P, NT, C], f32r, name="w_sb", tag="w")

            with ExitStack() as pkv:
                xcpool = pkv.enter_context(tc.tile_pool(name="xcpool", bufs=2))
                vstage = pkv.enter_context(tc.tile_pool(name="vstage", bufs=4))
                for jc in range(NCHK):
                    xck = xcpool.tile([P, NT, FBC], f32r, name="xck", tag="xc")
                    nc.sync.dma_start(
                        out=xck,
                        in_=xTb[:, jc * FBC:(jc + 1) * FBC].rearrange(
                            "(ci p) t -> p ci t", p=P
                        ),
                    )
                    # kT chunk: kT = Wk.T @ x (+bk), kept resident
                    for co in range(NT):
                        ps = psp.tile([P, FBC], f32, name="ps_k", tag="ps")
                        for ci in range(NT):
                            nc.tensor.matmul(
                                ps,
                                wk_sb[:, ci, co * P:(co + 1) * P],
                                xck[:, ci, :],
                                start=(ci == 0),
                                stop=(ci == NT - 1),
                            )
                        nc.scalar.activation(
                            out=kT_sb[:, co, jc * FBC:(jc + 1) * FBC],
                            in_=ps,
                            func=AFT.Identity,
                            bias=bk_sb[:, co:co + 1],
                        )
                    if jc == 0:
                        # Wv load deferred past chunk-0 K work so the
                        # startup-critical Wk/x transfers get the bandwidth
                        for ci in range(NT):
                            nc.gpsimd.dma_start(
                                out=wv_sb[:, ci, :],
                                in_=Wv[ci * P:(ci + 1) * P, :],
                            )
                    # v chunk: v = x @ Wv, staged to DRAM (bv folded in bo_t)
                    for jt in range(FBC // P):
                        for ch in range(NCH):
                            ps = psp.tile([P, CH], f32, name="ps_v", tag="ps")
                            for ci in range(NT):
                                nc.tensor.matmul(
                                    ps,
                                    xck[:, ci, jt * P:(jt + 1) * P],
                                    wv_sb[:, ci, ch * CH:(ch + 1) * CH],
                                    start=(ci == 0),
                                    stop=(ci == NT - 1),
                                )
                            vs = vstage.tile([P, CH], f32r, name="vs", tag="vs")
                            nc.vector.tensor_copy(vs, ps)
                            r0 = jc * FBC + jt * P
                            w = nc.sync.dma_start(
                                out=v_dram[r0:r0 + P, ch * CH:(ch + 1) * CH],
                                in_=vs,
                            )
                            v_w_insts.setdefault(r0 // P, []).append(w)

          # pass Q: qT = Wq.T @ xq (+bq); Wq streams as per-co panels
          with ExitStack() as pq0:
            wqp = pq0.enter_context(tc.tile_pool(name="wqp", bufs=2))
            xqpool = pq0.enter_context(tc.tile_pool(name="xqpool", bufs=1))
            xcqs = []
            for qc in range(TL // FBQ):
                xcq = xqpool.tile([P, NT, FBQ], f32r, name="xcq", tag=f"xcq{qc}")
                nc.gpsimd.dma_start(
                    out=xcq,
                    in_=xq[:, qc * FBQ:(qc + 1) * FBQ].rearrange(
                        "(ci p) t -> p ci t", p=P
                    ),
                )
                xcqs.append(xcq)
            for co in range(NT):
                wq_p = wqp.tile([P, NT, P], f32r, name="wq_p", tag="wqp")
                nc.gpsimd.dma_start(
                    out=wq_p,
                    in_=Wq[co].rearrange("(ci p) m -> p ci m", p=P),
                )
                for qc in range(TL // FBQ):
                    ps = psp.tile([P, FBQ], f32, name="ps_q", tag="ps")
                    for ci in range(NT):
                        nc.tensor.matmul(
                            ps,
                            wq_p[:, ci, :],
                            xcqs[qc][:, ci, :],
                            start=(ci == 0),
                            stop=(ci == NT - 1),
                        )
                    nc.scalar.activation(
                        out=qT_sb[:, co, qc * FBQ:(qc + 1) * FBQ],
                        in_=ps,
                        func=AFT.Identity,
                        bias=bq_sb[:, co:co + 1],
                    )
            # mask masters load after the Q-critical transfers
            nc.gpsimd.dma_start(out=mA_sb, in_=mA)
            nc.gpsimd.dma_start(out=mB_sb, in_=mB)

        # -------- phase 2: attention + output projection --------
        with ExitStack() as p2:
            probsp = p2.enter_context(tc.tile_pool(name="probsp", bufs=njB))
            vpanelp = p2.enter_context(tc.tile_pool(name="vpanelp", bufs=5))
            wop = p2.enter_context(tc.tile_pool(name="wop", bufs=4))
            attnp = p2.enter_context(tc.tile_pool(name="attnp", bufs=1))
            recp = p2.enter_context(tc.tile_pool(name="recp", bufs=2))
            ostagep = p2.enter_context(tc.tile_pool(name="ostagep", bufs=2))

            for a, (nj, j0m, Cm, m_sb) in enumerate(
                [(njA, 0, CA, mA_sb), (njB, NJB0, CB, mB_sb)]
            ):
                # v panels pre-issued: they fill during the scores loop
                vps = []
                for jt in range(nj):
                    vp = vpanelp.tile([P, C], f32r, name="vp", tag="vp")
                    vp_r = nc.sync.dma_start(
                        out=vp, in_=v_dram[jt * P:(jt + 1) * P, :]
                    )
                    for w in v_w_insts[jt]:
                        add_dep_helper(vp_r.ins, w.ins, reason="v_dram RAW")
                    vps.append(vp)

                # scores + exp + mask + denominator
                probs_tiles = []
                ps_den = psp.tile([1, FB], f32, name="ps_den", tag="ps")
                for jt in range(nj):
                    ps_s = psp.tile([P, FB], f32, name="ps_s", tag="ps")
                    for ci in range(NT):
                        nc.tensor.matmul(
                            ps_s,
                            kT_sb[:, ci, jt * P:(jt + 1) * P],
                            qT_sb[:, ci, a * FB:(a + 1) * FB],
                            start=(ci == 0),
                            stop=(ci == NT - 1),
                        )
                    pj = probsp.tile([P, FB], f32r, name="pj", tag="pj")
                    nc.scalar.activation(out=pj, in_=ps_s, func=AFT.Exp, scale=SC)
                    if jt >= j0m:  # earlier j-tiles are all-ones on every core
                        s0 = Cm - P * (jt - j0m)
                        nc.vector.tensor_mul(pj, pj, m_sb[:, s0:s0 + FB])
                    nc.tensor.matmul(
                        ps_den,
                        ones_sb,
                        pj,
                        start=(jt == 0),
                        stop=(jt == nj - 1),
                        skip_group_check=True,
                    )
                    probs_tiles.append(pj)

                # 1/denominator: quick copy releases the PSUM bank, then
                # the slow iterative reciprocal runs off the SBUF copy
                den_sb = recp.tile([1, FB], f32, name="den_sb", tag="den_sb")
                nc.scalar.copy(den_sb, ps_den)
                rrow = recp.tile([1, FB], f32, name="rrow", tag="rrow")
                nc.vector.reciprocal(rrow, den_sb)
                rec_w = nc.sync.dma_start(out=rec_dram[a:a + 1, :], in_=rrow)
                recipB = recp.tile([P, FB], f32, name="recipB", tag="recipB")
                rec_row = rec_dram[a, :]
                rec_bcast = bass.AP(
                    tensor=rec_row.tensor,
                    offset=rec_row.offset,
                    ap=[[0, P]] + [list(d) for d in rec_row.ap],
                )
                rec_r = nc.sync.dma_start(out=recipB, in_=rec_bcast)
                add_dep_helper(rec_r.ins, rec_w.ins, reason="rec_dram RAW")

                # PV: attnT[c, i] accumulated over j tiles
                ps_attn = [
                    psp.tile([P, FB], f32, name="ps_attn", tag="ps")
                    for _ in range(NT)
                ]
                for jt in range(nj):
                    for ct in range(NT):
                        nc.tensor.matmul(
                            ps_attn[ct],
                            vps[jt][:, ct * P:(ct + 1) * P],
                            probs_tiles[jt],
                            start=(jt == 0),
                            stop=(jt == nj - 1),
                            skip_group_check=True,
                        )
                attn_sb = attnp.tile([P, NT, FB], f32r, name="attn_sb", tag="attn")
                for ct in range(NT):
                    nc.vector.tensor_mul(attn_sb[:, ct, :], ps_attn[ct], recipB)

                # output projection (+ folded bv@Wo + bo bias)
                for co in range(NT):
                    wo_sb = wop.tile([P, NT, P], f32r, name="wo_sb", tag="wo")
                    nc.gpsimd.dma_start(
                        out=wo_sb,
                        in_=WoT[co].rearrange("(ci p) m -> p ci m", p=P),
                    )
                    ps_o = psp.tile([P, FB], f32, name="ps_o", tag="ps")
                    for ci in range(NT):
                        nc.tensor.matmul(
                            ps_o,
                            wo_sb[:, ci, :],
                            attn_sb[:, ci, :],
                            start=(ci == 0),
                            stop=(ci == NT - 1),
                        )
                    os_ = ostagep.tile([P, FB], f32, name="os_", tag="os")
                    nc.scalar.activation(
                        out=os_, in_=ps_o, func=AFT.Identity,
                        bias=bo_sb[:, co:co + 1],
                    )
                    nc.sync.dma_start(
                        out=outT[co * P:(co + 1) * P, a * FB:(a + 1) * FB],
                        in_=os_,
                    )


def build_program(T=T_, C=C_, num_cores=8):
    """Build and compile the SPMD Bass program."""
    from concourse import bacc, mybir
    import concourse.tile as tile

    f32 = mybir.dt.float32
    f32r = mybir.dt.float32r
    NT = C // P
    BLK = T // 4
    TL = 2 * BLK
    njA = (2 * BLK) // P
    njB = (4 * BLK) // P
    CA = P * (njA - 1)
    CB = P * (njB - 1 - njA)

    nc = bacc.Bacc(
        "TRN2", target_bir_lowering=False, debug=False, num_devices=num_cores
    )
    xTb = nc.dram_tensor("xTb", [C, T], f32r, kind="ExternalInput").ap()
    xq = nc.dram_tensor("xq", [C, TL], f32r, kind="ExternalInput").ap()
    Wk = nc.dram_tensor("WkT", [NT, C, P], f32r, kind="ExternalInput").ap()
    Wv = nc.dram_tensor("Wv", [C, C], f32r, kind="ExternalInput").ap()
    Wq = nc.dram_tensor("WqT", [NT, C, P], f32r, kind="ExternalInput").ap()
    WoT = nc.dram_tensor("WoT", [NT, C, P], f32r, kind="ExternalInput").ap()
    bq_t = nc.dram_tensor("bq_t", [P, NT], f32, kind="ExternalInput").ap()
    bk_t = nc.dram_tensor("bk_t", [P, NT], f32, kind="ExternalInput").ap()
    bo_t = nc.dram_tensor("bo_t", [P, NT], f32, kind="ExternalInput").ap()
    ones_d = nc.dram_tensor("ones_d", [P, 1], f32r, kind="ExternalInput").ap()
    mA = nc.dram_tensor("mA", [P, CA + BLK], f32r, kind="ExternalInput").ap()
    mB = nc.dram_tensor("mB", [P, CB + BLK], f32r, kind="ExternalInput").ap()
    v_dram = nc.dram_tensor("v_int", [T, C], f32r).ap()
    rec_dram = nc.dram_tensor("rec_int", [2, BLK], f32).ap()
    outT = nc.dram_tensor("outT", [C, TL], f32, kind="ExternalOutput").ap()

    aps = (xTb, xq, Wk, Wv, Wq, WoT, bq_t, bk_t, bo_t, ones_d, mA, mB,
           v_dram, rec_dram, outT)
    with tile.TileContext(nc) as tc:
        _emit(nc, tc, aps, T, C)
    nc.compile()
    return nc


def make_core_inputs(x, Wq, bq, Wk, bk, Wv, bv, Wo, bo, T=T_, C=C_):
    """Per-core input maps (list of 8 dicts) for the SPMD program."""
    f = np.float32
    NT = C // P
    BLK = T // 4
    njA = (2 * BLK) // P
    njB = (4 * BLK) // P
    CA = P * (njA - 1)
    CB = P * (njB - 1 - njA)

    x = np.asarray(x, f)
    Wq, Wk, Wv, Wo = (np.asarray(w, f) for w in (Wq, Wk, Wv, Wo))
    bq, bk, bv, bo = (np.asarray(b, f) for b in (bq, bk, bv, bo))

    def cotile(W):  # [C, C] -> [NT, C, P] with [t] = W[:, t*P:(t+1)*P]
        return np.ascontiguousarray(W.reshape(C, NT, P).transpose(1, 0, 2))

    WoT = cotile(Wo)
    WkT = cotile(Wk)
    WqT = cotile(Wq)
    bo_eff = (bv @ Wo + bo).astype(f)

    def tr(b):  # [C] -> [P, NT] with b_t[p, t] = b[t*P + p]
        return np.ascontiguousarray(b.reshape(NT, P).T)

    def mask(CC, i0, width):
        pp = np.arange(P, dtype=np.int64)[:, None]
        gg = np.arange(width, dtype=np.int64)[None, :]
        return np.ascontiguousarray((pp <= gg - CC + i0).astype(f))

    ones = np.ones((P, 1), f)

    maps = []
    for core in range(8):
        b, p = core // 2, core % 2
        lo, hi = (0, 3) if p == 0 else (1, 2)
        xTv = np.ascontiguousarray(x[b].T)  # [C, T]
        xqb = np.ascontiguousarray(
            np.concatenate(
                [xTv[:, lo * BLK:(lo + 1) * BLK], xTv[:, hi * BLK:(hi + 1) * BLK]],
                axis=1,
            )
        )
        maps.append(
            {
                "xTb": xTv,
                "xq": xqb,
                "WkT": WkT,
                "Wv": Wv,
                "WqT": WqT,
                "WoT": WoT,
                "bq_t": tr(bq),
                "bk_t": tr(bk),
                "bo_t": tr(bo_eff),
                "ones_d": ones,
                "mA": mask(CA, lo * BLK, CA + BLK),
                "mB": mask(CB + njA * P, hi * BLK, CB + BLK),
            }
        )
    return maps


def gather_output(results, T=T_, C=C_, B=B_):
    BLK = T // 4
    out = np.empty((B, T, C), np.float32)
    for core in range(8):
        b, p = core // 2, core % 2
        lo, hi = (0, 3) if p == 0 else (1, 2)
        oT = results[core]["outT"]
        out[b, lo * BLK:(lo + 1) * BLK] = oT[:, 0:BLK].T
        out[b, hi * BLK:(hi + 1) * BLK] = oT[:, BLK:2 * BLK].T
    return out


_NC_CACHE = {}


def kernel(x, Wq, bq, Wk, bk, Wv, bv, Wo, bo):
    from concourse.bass_utils import run_bass_kernel_spmd

    key = "full"
    if key not in _NC_CACHE:
        _NC_CACHE[key] = build_program()
    nc = _NC_CACHE[key]
    in_maps = make_core_inputs(x, Wq, bq, Wk, bk, Wv, bv, Wo, bo)
    res = run_bass_kernel_spmd(nc, in_maps, list(range(8))).results
    return gather_output(res)



# revision 6
# speedup vs baseline: 1.2762x; 1.2762x over previous
"""Causal full-d_model attention (B=4, T=2048, C=1024) on 8 Trainium2 cores.

Sharding: core = 2*b + p handles batch b and two 512-row sequence blocks,
p=0 -> blocks {0, 3}, p=1 -> blocks {1, 2} (pairing balances causal work).
K/V projections for the full sequence are computed redundantly on both
cores of a batch pair; the causal skip of upper-triangle score/PV tiles
pays back exactly that duplication, so per-core FLOPs equal an ideal
8-way split (~17.2 GFLOP).

All matmul operands are bf16 (same 1 col/cycle PE rate as f32r but half
the DMA/SBUF/LDWEIGHTS traffic; measured rel err ~5e-3 vs the fp32
reference); accumulation is fp32 in PSUM, softmax denominator/reciprocal
and biases stay fp32.

On-device layout is transposed ([feature, token]) so every matmul
contracts along the partition axis:
    qT/kT = W.T @ xT                       (projection)
    scoresT[j, i] = kT_slice.T @ qT        (j on partitions)
    attnT[c, i]  += v_slice.T @ probsT     (accumulate over j tiles)
    outT          = Wo_slice.T @ attnT
Phase order is Q -> K/V -> attention; every weight is host-pre-laid-out
so its load is one contiguous DMA, and all loads are prefetched at t=0
across four engine queues (sync/gpsimd/scalar/vector), so the PE never
waits on HBM.  V stays resident in SBUF (no DRAM staging).  Softmax is
unnormalized (scores ~ N(0,1), exp is safe); the denominator comes from
an M=1 ones-column matmul over the masked exp tiles, its reciprocal is
broadcast to 128 partitions with a K=1 ones matmul, and applied to attnT
at the PSUM->SBUF copy.  Causal masks arrive as per-core input data (a
shifted window into a master 0/1 pattern), so all 8 cores run one SPMD
program even though their absolute row offsets differ.
"""

import math

import numpy as np

P = 128          # SBUF partitions
B_, T_, C_ = 4, 2048, 1024


def _emit(nc, tc, aps, T, C):
    from concourse import mybir
    from contextlib import ExitStack

    AFT = mybir.ActivationFunctionType
    f32 = mybir.dt.float32
    bf16 = mybir.dt.bfloat16

    NT = C // P            # feature tiles (8)
    BLK = T // 4           # sequence block = i-slot width FB (512)
    TL = 2 * BLK           # local query tokens per core (1024)
    FB = BLK               # matmul moving free dim for i (512)
    FC = 512               # x chunk width (Q and K/V passes)
    NCHK = T // FC         # 4 K/V chunks
    NQC = TL // FC         # 2 Q chunks
    NJ = T // P            # 16 total j-tiles of v
    njA = (2 * BLK) // P   # padded j-tiles for slot A (8)
    njB = (4 * BLK) // P   # padded j-tiles for slot B (16)
    CA = P * (njA - 1)
    NJB0 = njA            # slot-B j-tiles < NJB0 are causally all-ones for
    CB = P * (njB - 1 - NJB0)   # every core; mask multiply is skipped there
    SC = 1.0 / math.sqrt(C)

    (xR, xqR, WqR, WkR, WvR, WoR, bq_t, bk_t, bo_t, ones_d, onesr_d,
     mA, mB, outT) = aps

    with ExitStack() as ctx:
        singles = ctx.enter_context(tc.tile_pool(name="singles", bufs=1))
        kpool = ctx.enter_context(tc.tile_pool(name="kpool", bufs=1))
        qpool = ctx.enter_context(tc.tile_pool(name="qpool", bufs=1))
        vpool = ctx.enter_context(tc.tile_pool(name="vpool", bufs=1))
        wpool = ctx.enter_context(tc.tile_pool(name="wpool", bufs=1))
        maskp = ctx.enter_context(tc.tile_pool(name="maskp", bufs=1))

        # ---- prefetch everything up front, spread across engine queues ----
        bq_sb = singles.tile([P, NT], f32, name="bq_sb")
        bk_sb = singles.tile([P, NT], f32, name="bk_sb")
        bo_sb = singles.tile([P, NT], f32, name="bo_sb")
        ones_sb = singles.tile([P, 1], bf16, name="ones_sb")
        onesr_sb = singles.tile([1, P], bf16, name="onesr_sb")
        nc.scalar.dma_start(out=bq_sb, in_=bq_t)
        nc.scalar.dma_start(out=bk_sb, in_=bk_t)
        nc.scalar.dma_start(out=bo_sb, in_=bo_t)
        nc.scalar.dma_start(out=ones_sb, in_=ones_d)
        nc.scalar.dma_start(out=onesr_sb, in_=onesr_d)

        # weights, one contiguous panel per co on gpsimd/scalar queues
        wk_sb = wpool.tile([P, NT, NT, P], bf16, name="wk_sb")
        wv_sb = wpool.tile([P, NT, C], bf16, name="wv_sb")
        wo_sb = wpool.tile([P, NT, NT, P], bf16, name="wo_sb")

        mA_sb = maskp.tile([P, CA + FB], bf16, name="mA_sb")
        mB_sb = maskp.tile([P, CB + FB], bf16, name="mB_sb")
        nc.gpsimd.dma_start(out=mA_sb, in_=mA)
        nc.gpsimd.dma_start(out=mB_sb, in_=mB)

        kT_sb = kpool.tile([P, NT, T], bf16, name="kT_sb")
        qT_sb = qpool.tile([P, NT, TL], bf16, name="qT_sb")
        v_sb = vpool.tile([P, NJ, C], bf16, name="v_sb")

        # x chunks: xq first, then the 4 K/V chunks, all on the sync queue
        with ExitStack() as p1:
            wqpool = p1.enter_context(tc.tile_pool(name="wqpool", bufs=1))
            xpool = p1.enter_context(tc.tile_pool(name="xpool", bufs=3))
            psp1 = p1.enter_context(
                tc.tile_pool(name="psp1", bufs=4, space="PSUM"))
            wq_sb = wqpool.tile([P, NT, NT, P], bf16, name="wq_sb")
            for co in range(NT):
                nc.gpsimd.dma_start(out=wq_sb[:, co], in_=WqR[:, co])
            for co in range(NT):
                nc.gpsimd.dma_start(out=wk_sb[:, co], in_=WkR[:, co])
            nc.scalar.dma_start(out=wv_sb, in_=WvR)
            nc.scalar.dma_start(out=wo_sb, in_=WoR)
            xqs = []
            for qc in range(NQC):
                xcq = xpool.tile([P, NT, FC], bf16, name="xcq", tag="xc")
                nc.sync.dma_start(out=xcq, in_=xqR[qc])
                xqs.append(xcq)

            # ---- pass Q: qT = Wq.T @ xq (+bq) ----
            for co in range(NT):
                for qc in range(NQC):
                    ps = psp1.tile([P, FC], f32, name="ps_q", tag="ps")
                    for ci in range(NT):
                        nc.tensor.matmul(
                            ps,
                            wq_sb[:, co, ci, :],
                            xqs[qc][:, ci, :],
                            start=(ci == 0),
                            stop=(ci == NT - 1),
                        )
                    nc.scalar.activation(
                        out=qT_sb[:, co, qc * FC:(qc + 1) * FC],
                        in_=ps,
                        func=AFT.Identity,
                        bias=bq_sb[:, co:co + 1],
                    )

            # ---- pass K+V per x chunk ----
            for jc in range(NCHK):
                xck = xpool.tile([P, NT, FC], bf16, name="xck", tag="xc")
                nc.sync.dma_start(out=xck, in_=xR[jc])
                for co in range(NT):
                    ps = psp1.tile([P, FC], f32, name="ps_k", tag="ps")
                    for ci in range(NT):
                        nc.tensor.matmul(
                            ps,
                            wk_sb[:, co, ci, :],
                            xck[:, ci, :],
                            start=(ci == 0),
                            stop=(ci == NT - 1),
                        )
                    nc.scalar.activation(
                        out=kT_sb[:, co, jc * FC:(jc + 1) * FC],
                        in_=ps,
                        func=AFT.Identity,
                        bias=bk_sb[:, co:co + 1],
                    )
                # v = x @ Wv  (bv folded into bo_t on host)
                for jt in range(FC // P):
                    for ch in range(2):
                        ps = psp1.tile([P, 512], f32, name="ps_v", tag="ps")
                        for ci in range(NT):
                            nc.tensor.matmul(
                                ps,
                                xck[:, ci, jt * P:(jt + 1) * P],
                                wv_sb[:, ci, ch * 512:(ch + 1) * 512],
                                start=(ci == 0),
                                stop=(ci == NT - 1),
                            )
                        nc.vector.tensor_copy(
                            v_sb[:, jc * (FC // P) + jt,
                                 ch * 512:(ch + 1) * 512],
                            ps,
                        )

        # -------- phase 2: attention + output projection --------
        with ExitStack() as p2:
            probsp = p2.enter_context(tc.tile_pool(name="probsp", bufs=njA + njB))
            attnp = p2.enter_context(tc.tile_pool(name="attnp", bufs=1))
            recp = p2.enter_context(tc.tile_pool(name="recp", bufs=2))
            ostagep = p2.enter_context(tc.tile_pool(name="ostagep", bufs=2))
            pscore = p2.enter_context(
                tc.tile_pool(name="pscore", bufs=2, space="PSUM"))
            pden = p2.enter_context(
                tc.tile_pool(name="pden", bufs=1, space="PSUM"))
            prec = p2.enter_context(
                tc.tile_pool(name="prec", bufs=1, space="PSUM"))
            pattn = p2.enter_context(
                tc.tile_pool(name="pattn", bufs=2, space="PSUM"))
            pout = p2.enter_context(
                tc.tile_pool(name="pout", bufs=2, space="PSUM"))

            for a, (nj, j0m, Cm, m_sb) in enumerate(
                [(njA, 0, CA, mA_sb), (njB, NJB0, CB, mB_sb)]
            ):
                # scores + exp + mask + denominator
                probs_tiles = []
                ps_den = pden.tile([1, FB], f32, name="ps_den", tag="ps_den")
                for jt in range(nj):
                    ps_s = pscore.tile([P, FB], f32, name="ps_s", tag="ps_s")
                    for ci in range(NT):
                        nc.tensor.matmul(
                            ps_s,
                            kT_sb[:, ci, jt * P:(jt + 1) * P],
                            qT_sb[:, ci, a * FB:(a + 1) * FB],
                            start=(ci == 0),
                            stop=(ci == NT - 1),
                        )
                    pj = probsp.tile([P, FB], bf16, name="pj", tag="pj")
                    nc.scalar.activation(out=pj, in_=ps_s, func=AFT.Exp, scale=SC)
                    if jt >= j0m:  # earlier j-tiles are all-ones on every core
                        s0 = Cm - P * (jt - j0m)
                        nc.vector.tensor_mul(pj, pj, m_sb[:, s0:s0 + FB])
                    nc.tensor.matmul(
                        ps_den,
                        ones_sb,
                        pj,
                        start=(jt == 0),
                        stop=(jt == nj - 1),
                        skip_group_check=True,
                    )
                    probs_tiles.append(pj)

                # 1/denominator: quick copy releases the PSUM bank, fast
                # approx reciprocal; a K=1 ones matmul broadcasts it to all
                # 128 partitions (no DRAM round trip).  The broadcast matmul
                # is emitted after PV pass 0 so the PE queue never waits on
                # the scalar/vector reciprocal chain.
                den_sb = recp.tile([1, FB], f32, name="den_sb", tag="den_sb")
                nc.scalar.copy(den_sb, ps_den)
                rrow = recp.tile([1, FB], f32, name="rrow", tag="rrow")
                nc.vector.reciprocal_approx_fast(rrow, den_sb)
                rrow_b = recp.tile([1, FB], bf16, name="rrow_b", tag="rrow_b")
                nc.vector.tensor_copy(rrow_b, rrow)
                rec_sb = recp.tile([P, FB], f32, name="rec_sb", tag="rec_sb")

                # PV: attnT[c, i] accumulated over j tiles, two banks at a
                # time (4 passes over the probs tiles) so slot B scores can
                # overlap slot A PV on the free score banks
                attn_sb = attnp.tile([P, NT, FB], bf16, name="attn_sb",
                                     tag="attn")
                for half in range(4):
                    ps_attn = [
                        pattn.tile([P, FB], f32, name="ps_attn", tag="ps_a")
                        for _ in range(2)
                    ]
                    for jt in range(nj):
                        for c2 in range(2):
                            ct = half * 2 + c2
                            nc.tensor.matmul(
                                ps_attn[c2],
                                v_sb[:, jt, ct * P:(ct + 1) * P],
                                probs_tiles[jt],
                                start=(jt == 0),
                                stop=(jt == nj - 1),
                                skip_group_check=True,
                            )
                    if half == 0:
                        ps_rec = prec.tile([P, FB], f32, name="ps_rec",
                                           tag="ps_rec")
                        nc.tensor.matmul(ps_rec, onesr_sb, rrow_b,
                                         start=True, stop=True)
                        nc.scalar.copy(rec_sb, ps_rec)
                    for c2 in range(2):
                        ct = half * 2 + c2
                        nc.vector.tensor_mul(
                            attn_sb[:, ct, :], ps_attn[c2], rec_sb)

                # output projection (+ folded bv@Wo + bo bias)
                for co in range(NT):
                    ps_o = pout.tile([P, FB], f32, name="ps_o", tag="ps_o")
                    for ci in range(NT):
                        nc.tensor.matmul(
                            ps_o,
                            wo_sb[:, co, ci, :],
                            attn_sb[:, ci, :],
                            start=(ci == 0),
                            stop=(ci == NT - 1),
                        )
                    os_ = ostagep.tile([P, FB], bf16, name="os_", tag="os")
                    nc.scalar.activation(
                        out=os_, in_=ps_o, func=AFT.Identity,
                        bias=bo_sb[:, co:co + 1],
                    )
                    nc.sync.dma_start(
                        out=outT[co * P:(co + 1) * P, a * FB:(a + 1) * FB],
                        in_=os_,
                    )


def build_program(T=T_, C=C_, num_cores=8):
    """Build and compile the SPMD Bass program."""
    from concourse import bacc, mybir
    import concourse.tile as tile

    f32 = mybir.dt.float32
    bf16 = mybir.dt.bfloat16
    NT = C // P
    BLK = T // 4
    TL = 2 * BLK
    FC = 512
    njA = (2 * BLK) // P
    njB = (4 * BLK) // P
    CA = P * (njA - 1)
    CB = P * (njB - 1 - njA)

    nc = bacc.Bacc(
        "TRN2", target_bir_lowering=False, debug=False, num_devices=num_cores
    )
    xR = nc.dram_tensor("xR", [T // FC, P, NT, FC], bf16,
                        kind="ExternalInput").ap()
    xqR = nc.dram_tensor("xqR", [TL // FC, P, NT, FC], bf16,
                         kind="ExternalInput").ap()
    WqR = nc.dram_tensor("WqR", [P, NT, NT, P], bf16, kind="ExternalInput").ap()
    WkR = nc.dram_tensor("WkR", [P, NT, NT, P], bf16, kind="ExternalInput").ap()
    WvR = nc.dram_tensor("WvR", [P, NT, C], bf16, kind="ExternalInput").ap()
    WoR = nc.dram_tensor("WoR", [P, NT, NT, P], bf16, kind="ExternalInput").ap()
    bq_t = nc.dram_tensor("bq_t", [P, NT], f32, kind="ExternalInput").ap()
    bk_t = nc.dram_tensor("bk_t", [P, NT], f32, kind="ExternalInput").ap()
    bo_t = nc.dram_tensor("bo_t", [P, NT], f32, kind="ExternalInput").ap()
    ones_d = nc.dram_tensor("ones_d", [P, 1], bf16, kind="ExternalInput").ap()
    onesr_d = nc.dram_tensor("onesr_d", [1, P], bf16, kind="ExternalInput").ap()
    mA = nc.dram_tensor("mA", [P, CA + BLK], bf16, kind="ExternalInput").ap()
    mB = nc.dram_tensor("mB", [P, CB + BLK], bf16, kind="ExternalInput").ap()
    outT = nc.dram_tensor("outT", [C, TL], bf16, kind="ExternalOutput").ap()

    aps = (xR, xqR, WqR, WkR, WvR, WoR, bq_t, bk_t, bo_t, ones_d, onesr_d,
           mA, mB, outT)
    with tile.TileContext(nc) as tc:
        _emit(nc, tc, aps, T, C)
    nc.compile()
    return nc


def make_core_inputs(x, Wq, bq, Wk, bk, Wv, bv, Wo, bo, T=T_, C=C_):
    """Per-core input maps (list of 8 dicts) for the SPMD program."""
    import ml_dtypes

    f = np.float32
    b16 = ml_dtypes.bfloat16
    NT = C // P
    BLK = T // 4
    FC = 512
    njA = (2 * BLK) // P
    njB = (4 * BLK) // P
    CA = P * (njA - 1)
    CB = P * (njB - 1 - njA)

    x = np.asarray(x, f)
    Wq, Wk, Wv, Wo = (np.asarray(w, f) for w in (Wq, Wk, Wv, Wo))
    bq, bk, bv, bo = (np.asarray(b, f) for b in (bq, bk, bv, bo))

    def cotile(W):  # [C, C] -> [P, NT(co), NT(ci), P]: W[ci*P+p, co*P+m]
        return np.ascontiguousarray(
            W.reshape(NT, P, NT, P).transpose(1, 2, 0, 3)).astype(b16)

    WqRl = cotile(Wq)
    WkRl = cotile(Wk)
    WoRl = cotile(Wo)
    WvRl = np.ascontiguousarray(Wv.reshape(NT, P, C).transpose(1, 0, 2)
                                ).astype(b16)
    bo_eff = (bv @ Wo + bo).astype(f)

    def tr(b):  # [C] -> [P, NT] with b_t[p, t] = b[t*P + p]
        return np.ascontiguousarray(b.reshape(NT, P).T)

    def mask(CC, i0, width):
        pp = np.arange(P, dtype=np.int64)[:, None]
        gg = np.arange(width, dtype=np.int64)[None, :]
        return np.ascontiguousarray((pp <= gg - CC + i0).astype(b16))

    ones = np.ones((P, 1), b16)
    onesr = np.ones((1, P), b16)

    def chunked(xT):  # [C, W] -> [W//FC, P, NT, FC]: xT[ci*P+p, c*FC+t]
        W = xT.shape[1]
        return np.ascontiguousarray(
            xT.reshape(NT, P, W // FC, FC).transpose(2, 1, 0, 3)).astype(b16)

    maps = []
    for core in range(8):
        b, p = core // 2, core % 2
        lo, hi = (0, 3) if p == 0 else (1, 2)
        xTv = np.ascontiguousarray(x[b].T)  # [C, T]
        xqb = np.concatenate(
            [xTv[:, lo * BLK:(lo + 1) * BLK], xTv[:, hi * BLK:(hi + 1) * BLK]],
            axis=1,
        )
        maps.append(
            {
                "xR": chunked(xTv),
                "xqR": chunked(xqb),
                "WqR": WqRl,
                "WkR": WkRl,
                "WvR": WvRl,
                "WoR": WoRl,
                "bq_t": tr(bq),
                "bk_t": tr(bk),
                "bo_t": tr(bo_eff),
                "ones_d": ones,
                "onesr_d": onesr,
                "mA": mask(CA, lo * BLK, CA + BLK),
                "mB": mask(CB + njA * P, hi * BLK, CB + BLK),
            }
        )
    return maps


def gather_output(results, T=T_, C=C_, B=B_):
    BLK = T // 4
    out = np.empty((B, T, C), np.float32)
    for core in range(8):
        b, p = core // 2, core % 2
        lo, hi = (0, 3) if p == 0 else (1, 2)
        oT = np.asarray(results[core]["outT"], np.float32)
        out[b, lo * BLK:(lo + 1) * BLK] = oT[:, 0:BLK].T
        out[b, hi * BLK:(hi + 1) * BLK] = oT[:, BLK:2 * BLK].T
    return out


_NC_CACHE = {}


def kernel(x, Wq, bq, Wk, bk, Wv, bv, Wo, bo):
    from concourse.bass_utils import run_bass_kernel_spmd

    key = "full"
    if key not in _NC_CACHE:
        _NC_CACHE[key] = build_program()
    nc = _NC_CACHE[key]
    in_maps = make_core_inputs(x, Wq, bq, Wk, bk, Wv, bv, Wo, bo)
    res = run_bass_kernel_spmd(nc, in_maps, list(range(8))).results
    return gather_output(res)


# revision 10
# speedup vs baseline: 1.3020x; 1.0202x over previous
"""Causal full-d_model attention (B=4, T=2048, C=1024) on 8 Trainium2 cores.

Sharding: core = 2*b + p handles batch b and two 512-row sequence blocks,
p=0 -> blocks {0, 3}, p=1 -> blocks {1, 2} (pairing balances causal work).
K/V projections for the full sequence are computed redundantly on both
cores of a batch pair; the causal skip of upper-triangle score/PV tiles
pays back exactly that duplication, so per-core FLOPs equal an ideal
8-way split (~17.2 GFLOP).

All matmul operands are bf16 (same 1 col/cycle PE rate as f32r but half
the DMA/SBUF/LDWEIGHTS traffic; measured rel err ~5e-3 vs the fp32
reference); accumulation is fp32 in PSUM, softmax denominator/reciprocal
and biases stay fp32.

On-device layout is transposed ([feature, token]) so every matmul
contracts along the partition axis:
    qT/kT = W.T @ xT                       (projection)
    scoresT[j, i] = kT_slice.T @ qT        (j on partitions)
    attnT[c, i]  += v_slice.T @ probsT     (accumulate over j tiles)
    outT          = Wo_slice.T @ attnT
Phase order is Q -> K/V -> attention; every weight is host-pre-laid-out
so its load is one contiguous DMA, and all loads are prefetched at t=0
across four engine queues (sync/gpsimd/scalar/vector), so the PE never
waits on HBM.  V stays resident in SBUF (no DRAM staging).  Softmax is
unnormalized (scores ~ N(0,1), exp is safe); the denominator comes from
an M=1 ones-column matmul over the masked exp tiles, its reciprocal is
broadcast to 128 partitions with a K=1 ones matmul, and applied to attnT
at the PSUM->SBUF copy.  Causal masks arrive as per-core input data (a
shifted window into a master 0/1 pattern), so all 8 cores run one SPMD
program even though their absolute row offsets differ.
"""

import math

import numpy as np

P = 128          # SBUF partitions
B_, T_, C_ = 4, 2048, 1024


def _emit(nc, tc, aps, T, C):
    from concourse import mybir
    from concourse.tile_rust import add_dep_helper
    from contextlib import ExitStack

    AFT = mybir.ActivationFunctionType
    f32 = mybir.dt.float32
    bf16 = mybir.dt.bfloat16

    NT = C // P            # feature tiles (8)
    BLK = T // 4           # sequence block = i-slot width FB (512)
    TL = 2 * BLK           # local query tokens per core (1024)
    FB = BLK               # matmul moving free dim for i (512)
    FC = 512               # x chunk width (Q and K/V passes)
    NCHK = T // FC         # 4 K/V chunks
    NQC = TL // FC         # 2 Q chunks
    NJ = T // P            # 16 total j-tiles of v
    njA = (2 * BLK) // P   # padded j-tiles for slot A (8)
    njB = (4 * BLK) // P   # padded j-tiles for slot B (16)
    CA = P * (njA - 1)
    NJB0 = njA            # slot-B j-tiles < NJB0 are causally all-ones for
    CB = P * (njB - 1 - NJB0)   # every core; mask multiply is skipped there
    SC = 1.0 / math.sqrt(C)

    (xR, xqR, WqR, WkR, WvR, WoR, bq_t, bk_t, bo_t, ones_d, onesr_d,
     mA, mB, outT) = aps

    with ExitStack() as ctx:
        singles = ctx.enter_context(tc.tile_pool(name="singles", bufs=1))
        kpool = ctx.enter_context(tc.tile_pool(name="kpool", bufs=1))
        qpool = ctx.enter_context(tc.tile_pool(name="qpool", bufs=1))
        vpool = ctx.enter_context(tc.tile_pool(name="vpool", bufs=1))
        wpool = ctx.enter_context(tc.tile_pool(name="wpool", bufs=1))
        maskp = ctx.enter_context(tc.tile_pool(name="maskp", bufs=1))

        # ---- prefetch everything up front, spread across engine queues ----
        bq_sb = singles.tile([P, NT], f32, name="bq_sb")
        bk_sb = singles.tile([P, NT], f32, name="bk_sb")
        bo_sb = singles.tile([P, NT], f32, name="bo_sb")
        ones_sb = singles.tile([P, 1], bf16, name="ones_sb")
        onesr_sb = singles.tile([1, P], bf16, name="onesr_sb")
        nc.scalar.dma_start(out=bq_sb, in_=bq_t)
        nc.scalar.dma_start(out=bk_sb, in_=bk_t)
        nc.scalar.dma_start(out=bo_sb, in_=bo_t)
        nc.scalar.dma_start(out=ones_sb, in_=ones_d)
        nc.scalar.dma_start(out=onesr_sb, in_=onesr_d)

        # weights, one contiguous panel per co on gpsimd/scalar queues
        wk_sb = wpool.tile([P, NT, NT, P], bf16, name="wk_sb")
        wv_sb = wpool.tile([P, NT, C], bf16, name="wv_sb")
        wo_sb = wpool.tile([P, NT, NT, P], bf16, name="wo_sb")

        mA_sb = maskp.tile([P, CA + FB], bf16, name="mA_sb")
        mB_sb = maskp.tile([P, CB + FB], bf16, name="mB_sb")

        kT_sb = kpool.tile([P, NT, T], bf16, name="kT_sb")
        qT_sb = qpool.tile([P, NT, TL], bf16, name="qT_sb")
        v_sb = vpool.tile([P, NJ, C], bf16, name="v_sb")

        # x chunks: xq first, then the 4 K/V chunks, all on the sync queue
        with ExitStack() as p1:
            wqpool = p1.enter_context(tc.tile_pool(name="wqpool", bufs=1))
            xpool = p1.enter_context(tc.tile_pool(name="xpool", bufs=3))
            psp1 = p1.enter_context(
                tc.tile_pool(name="psp1", bufs=4, space="PSUM"))
            wq_sb = wqpool.tile([P, NT, NT, P], bf16, name="wq_sb")
            for co in range(NT):
                nc.gpsimd.dma_start(out=wq_sb[:, co], in_=WqR[:, co])
            for co in range(NT):
                nc.gpsimd.dma_start(out=wk_sb[:, co], in_=WkR[:, co])
            # masks behind wq/wk on the gpsimd queue; wv/wo transfers are
            # held back (dep added below) so the startup-critical wq/xq
            # loads get the full HBM bandwidth
            nc.gpsimd.dma_start(out=mA_sb, in_=mA)
            nc.gpsimd.dma_start(out=mB_sb, in_=mB)
            wv_w = nc.scalar.dma_start(out=wv_sb, in_=WvR)
            nc.scalar.dma_start(out=wo_sb, in_=WoR)
            xqs = []
            for qc in range(NQC):
                xcq = xpool.tile([P, NT, FC], bf16, name="xcq", tag="xc")
                nc.sync.dma_start(out=xcq, in_=xqR[qc])
                xqs.append(xcq)

            # ---- pass Q: qT = Wq.T @ xq (+bq) ----
            mm0 = None
            for co in range(NT):
                for qc in range(NQC):
                    ps = psp1.tile([P, FC], f32, name="ps_q", tag="ps")
                    for ci in range(NT):
                        mm = nc.tensor.matmul(
                            ps,
                            wq_sb[:, co, ci, :],
                            xqs[qc][:, ci, :],
                            start=(ci == 0),
                            stop=(ci == NT - 1),
                        )
                        if mm0 is None:
                            mm0 = mm
                            add_dep_helper(wv_w.ins, mm0.ins,
                                           reason="delay wv load")
                    nc.scalar.activation(
                        out=qT_sb[:, co, qc * FC:(qc + 1) * FC],
                        in_=ps,
                        func=AFT.Identity,
                        bias=bq_sb[:, co:co + 1],
                    )

            # ---- pass K+V per x chunk ----
            for jc in range(NCHK):
                xck = xpool.tile([P, NT, FC], bf16, name="xck", tag="xc")
                nc.sync.dma_start(out=xck, in_=xR[jc])
                for co in range(NT):
                    ps = psp1.tile([P, FC], f32, name="ps_k", tag="ps")
                    for ci in range(NT):
                        nc.tensor.matmul(
                            ps,
                            wk_sb[:, co, ci, :],
                            xck[:, ci, :],
                            start=(ci == 0),
                            stop=(ci == NT - 1),
                        )
                    nc.scalar.activation(
                        out=kT_sb[:, co, jc * FC:(jc + 1) * FC],
                        in_=ps,
                        func=AFT.Identity,
                        bias=bk_sb[:, co:co + 1],
                    )
                # v = x @ Wv  (bv folded into bo_t on host)
                for jt in range(FC // P):
                    for ch in range(2):
                        ps = psp1.tile([P, 512], f32, name="ps_v", tag="ps")
                        for ci in range(NT):
                            nc.tensor.matmul(
                                ps,
                                xck[:, ci, jt * P:(jt + 1) * P],
                                wv_sb[:, ci, ch * 512:(ch + 1) * 512],
                                start=(ci == 0),
                                stop=(ci == NT - 1),
                            )
                        nc.vector.tensor_copy(
                            v_sb[:, jc * (FC // P) + jt,
                                 ch * 512:(ch + 1) * 512],
                            ps,
                        )

        # -------- phase 2: attention + output projection --------
        with ExitStack() as p2:
            probsp = p2.enter_context(tc.tile_pool(name="probsp", bufs=njA + njB))
            attnp = p2.enter_context(tc.tile_pool(name="attnp", bufs=2))
            recp = p2.enter_context(tc.tile_pool(name="recp", bufs=2))
            ostagep = p2.enter_context(tc.tile_pool(name="ostagep", bufs=2))
            pscore = p2.enter_context(
                tc.tile_pool(name="pscore", bufs=2, space="PSUM"))
            pden = p2.enter_context(
                tc.tile_pool(name="pden", bufs=1, space="PSUM"))
            prec = p2.enter_context(
                tc.tile_pool(name="prec", bufs=1, space="PSUM"))
            pattn = p2.enter_context(
                tc.tile_pool(name="pattn", bufs=2, space="PSUM"))
            pout = p2.enter_context(
                tc.tile_pool(name="pout", bufs=2, space="PSUM"))

            attns = []
            for a, (nj, j0m, Cm, m_sb) in enumerate(
                [(njA, 0, CA, mA_sb), (njB, NJB0, CB, mB_sb)]
            ):
                # scores + exp + mask + denominator
                probs_tiles = []
                ps_den = pden.tile([1, FB], f32, name="ps_den", tag="ps_den")
                for jt in range(nj):
                    ps_s = pscore.tile([P, FB], f32, name="ps_s", tag="ps_s")
                    for ci in range(NT):
                        nc.tensor.matmul(
                            ps_s,
                            kT_sb[:, ci, jt * P:(jt + 1) * P],
                            qT_sb[:, ci, a * FB:(a + 1) * FB],
                            start=(ci == 0),
                            stop=(ci == NT - 1),
                        )
                    pj = probsp.tile([P, FB], bf16, name="pj", tag="pj")
                    nc.scalar.activation(out=pj, in_=ps_s, func=AFT.Exp, scale=SC)
                    if jt >= j0m:  # earlier j-tiles are all-ones on every core
                        s0 = Cm - P * (jt - j0m)
                        nc.vector.tensor_mul(pj, pj, m_sb[:, s0:s0 + FB])
                    nc.tensor.matmul(
                        ps_den,
                        ones_sb,
                        pj,
                        start=(jt == 0),
                        stop=(jt == nj - 1),
                        skip_group_check=True,
                    )
                    probs_tiles.append(pj)

                # 1/denominator: quick copy releases the PSUM bank, fast
                # approx reciprocal; a K=1 ones matmul broadcasts it to all
                # 128 partitions (no DRAM round trip).  The broadcast matmul
                # is emitted after PV pass 0 so the PE queue never waits on
                # the scalar/vector reciprocal chain.
                den_sb = recp.tile([1, FB], f32, name="den_sb", tag="den_sb")
                nc.scalar.copy(den_sb, ps_den)
                rrow = recp.tile([1, FB], f32, name="rrow", tag="rrow")
                nc.vector.reciprocal_approx_fast(rrow, den_sb)
                rrow_b = recp.tile([1, FB], bf16, name="rrow_b", tag="rrow_b")
                nc.vector.tensor_copy(rrow_b, rrow)
                rec_sb = recp.tile([P, FB], f32, name="rec_sb", tag="rec_sb")

                # PV: attnT[c, i] accumulated over j tiles, two banks at a
                # time (4 passes over the probs tiles) so slot B scores can
                # overlap slot A PV on the free score banks
                attn_sb = attnp.tile([P, NT, FB], bf16, name="attn_sb",
                                     tag="attn")
                for half in range(4):
                    ps_attn = [
                        pattn.tile([P, FB], f32, name="ps_attn", tag="ps_a")
                        for _ in range(2)
                    ]
                    for jt in range(nj):
                        for c2 in range(2):
                            ct = half * 2 + c2
                            nc.tensor.matmul(
                                ps_attn[c2],
                                v_sb[:, jt, ct * P:(ct + 1) * P],
                                probs_tiles[jt],
                                start=(jt == 0),
                                stop=(jt == nj - 1),
                                skip_group_check=True,
                            )
                    if half == 0:
                        ps_rec = prec.tile([P, FB], f32, name="ps_rec",
                                           tag="ps_rec")
                        nc.tensor.matmul(ps_rec, onesr_sb, rrow_b,
                                         start=True, stop=True)
                        nc.scalar.copy(rec_sb, ps_rec)
                    for c2 in range(2):
                        ct = half * 2 + c2
                        nc.vector.tensor_mul(
                            attn_sb[:, ct, :], ps_attn[c2], rec_sb)
                attns.append(attn_sb)

            # output projections last (+ folded bv@Wo + bo bias): slot A's
            # runs while slot B's attn muls finish, so the PE never waits
            for a in range(2):
                attn_sb = attns[a]
                for co in range(NT):
                    ps_o = pout.tile([P, FB], f32, name="ps_o", tag="ps_o")
                    for ci in range(NT):
                        nc.tensor.matmul(
                            ps_o,
                            wo_sb[:, co, ci, :],
                            attn_sb[:, ci, :],
                            start=(ci == 0),
                            stop=(ci == NT - 1),
                        )
                    os_ = ostagep.tile([P, FB], bf16, name="os_", tag="os")
                    nc.scalar.activation(
                        out=os_, in_=ps_o, func=AFT.Identity,
                        bias=bo_sb[:, co:co + 1],
                    )
                    nc.sync.dma_start(
                        out=outT[co * P:(co + 1) * P, a * FB:(a + 1) * FB],
                        in_=os_,
                    )


def build_program(T=T_, C=C_, num_cores=8):
    """Build and compile the SPMD Bass program."""
    from concourse import bacc, mybir
    import concourse.tile as tile

    f32 = mybir.dt.float32
    bf16 = mybir.dt.bfloat16
    NT = C // P
    BLK = T // 4
    TL = 2 * BLK
    FC = 512
    njA = (2 * BLK) // P
    njB = (4 * BLK) // P
    CA = P * (njA - 1)
    CB = P * (njB - 1 - njA)

    nc = bacc.Bacc(
        "TRN2", target_bir_lowering=False, debug=False, num_devices=num_cores
    )
    xR = nc.dram_tensor("xR", [T // FC, P, NT, FC], bf16,
                        kind="ExternalInput").ap()
    xqR = nc.dram_tensor("xqR", [TL // FC, P, NT, FC], bf16,
                         kind="ExternalInput").ap()
    WqR = nc.dram_tensor("WqR", [P, NT, NT, P], bf16, kind="ExternalInput").ap()
    WkR = nc.dram_tensor("WkR", [P, NT, NT, P], bf16, kind="ExternalInput").ap()
    WvR = nc.dram_tensor("WvR", [P, NT, C], bf16, kind="ExternalInput").ap()
    WoR = nc.dram_tensor("WoR", [P, NT, NT, P], bf16, kind="ExternalInput").ap()
    bq_t = nc.dram_tensor("bq_t", [P, NT], f32, kind="ExternalInput").ap()
    bk_t = nc.dram_tensor("bk_t", [P, NT], f32, kind="ExternalInput").ap()
    bo_t = nc.dram_tensor("bo_t", [P, NT], f32, kind="ExternalInput").ap()
    ones_d = nc.dram_tensor("ones_d", [P, 1], bf16, kind="ExternalInput").ap()
    onesr_d = nc.dram_tensor("onesr_d", [1, P], bf16, kind="ExternalInput").ap()
    mA = nc.dram_tensor("mA", [P, CA + BLK], bf16, kind="ExternalInput").ap()
    mB = nc.dram_tensor("mB", [P, CB + BLK], bf16, kind="ExternalInput").ap()
    outT = nc.dram_tensor("outT", [C, TL], bf16, kind="ExternalOutput").ap()

    aps = (xR, xqR, WqR, WkR, WvR, WoR, bq_t, bk_t, bo_t, ones_d, onesr_d,
           mA, mB, outT)
    with tile.TileContext(nc) as tc:
        _emit(nc, tc, aps, T, C)
    nc.compile()
    return nc


def make_core_inputs(x, Wq, bq, Wk, bk, Wv, bv, Wo, bo, T=T_, C=C_):
    """Per-core input maps (list of 8 dicts) for the SPMD program."""
    import ml_dtypes

    f = np.float32
    b16 = ml_dtypes.bfloat16
    NT = C // P
    BLK = T // 4
    FC = 512
    njA = (2 * BLK) // P
    njB = (4 * BLK) // P
    CA = P * (njA - 1)
    CB = P * (njB - 1 - njA)

    x = np.asarray(x, f)
    Wq, Wk, Wv, Wo = (np.asarray(w, f) for w in (Wq, Wk, Wv, Wo))
    bq, bk, bv, bo = (np.asarray(b, f) for b in (bq, bk, bv, bo))

    def cotile(W):  # [C, C] -> [P, NT(co), NT(ci), P]: W[ci*P+p, co*P+m]
        return np.ascontiguousarray(
            W.reshape(NT, P, NT, P).transpose(1, 2, 0, 3)).astype(b16)

    WqRl = cotile(Wq)
    WkRl = cotile(Wk)
    WoRl = cotile(Wo)
    WvRl = np.ascontiguousarray(Wv.reshape(NT, P, C).transpose(1, 0, 2)
                                ).astype(b16)
    bo_eff = (bv @ Wo + bo).astype(f)

    def tr(b):  # [C] -> [P, NT] with b_t[p, t] = b[t*P + p]
        return np.ascontiguousarray(b.reshape(NT, P).T)

    def mask(CC, i0, width):
        pp = np.arange(P, dtype=np.int64)[:, None]
        gg = np.arange(width, dtype=np.int64)[None, :]
        return np.ascontiguousarray((pp <= gg - CC + i0).astype(b16))

    ones = np.ones((P, 1), b16)
    onesr = np.ones((1, P), b16)

    def chunked(xT):  # [C, W] -> [W//FC, P, NT, FC]: xT[ci*P+p, c*FC+t]
        W = xT.shape[1]
        return np.ascontiguousarray(
            xT.reshape(NT, P, W // FC, FC).transpose(2, 1, 0, 3)).astype(b16)

    maps = []
    for core in range(8):
        b, p = core // 2, core % 2
        lo, hi = (0, 3) if p == 0 else (1, 2)
        xTv = np.ascontiguousarray(x[b].T)  # [C, T]
        xqb = np.concatenate(
            [xTv[:, lo * BLK:(lo + 1) * BLK], xTv[:, hi * BLK:(hi + 1) * BLK]],
            axis=1,
        )
        maps.append(
            {
                "xR": chunked(xTv),
                "xqR": chunked(xqb),
                "WqR": WqRl,
                "WkR": WkRl,
                "WvR": WvRl,
                "WoR": WoRl,
                "bq_t": tr(bq),
                "bk_t": tr(bk),
                "bo_t": tr(bo_eff),
                "ones_d": ones,
                "onesr_d": onesr,
                "mA": mask(CA, lo * BLK, CA + BLK),
                "mB": mask(CB + njA * P, hi * BLK, CB + BLK),
            }
        )
    return maps


def gather_output(results, T=T_, C=C_, B=B_):
    BLK = T // 4
    out = np.empty((B, T, C), np.float32)
    for core in range(8):
        b, p = core // 2, core % 2
        lo, hi = (0, 3) if p == 0 else (1, 2)
        oT = np.asarray(results[core]["outT"], np.float32)
        out[b, lo * BLK:(lo + 1) * BLK] = oT[:, 0:BLK].T
        out[b, hi * BLK:(hi + 1) * BLK] = oT[:, BLK:2 * BLK].T
    return out


_NC_CACHE = {}


def kernel(x, Wq, bq, Wk, bk, Wv, bv, Wo, bo):
    from concourse.bass_utils import run_bass_kernel_spmd

    key = "full"
    if key not in _NC_CACHE:
        _NC_CACHE[key] = build_program()
    nc = _NC_CACHE[key]
    in_maps = make_core_inputs(x, Wq, bq, Wk, bk, Wv, bv, Wo, bo)
    res = run_bass_kernel_spmd(nc, in_maps, list(range(8))).results
    return gather_output(res)
